# revision 3
# baseline (speedup 1.0000x reference)
"""Trainium2 Bass kernel for nn_BiologicalMemory (retrieval_knn).

Full-input contract: kernel(**inputs) takes the complete unsharded inputs and
returns the complete [4096] output. Internally shards across 8 NeuronCores:
  - memory_bank / importance / age row-sharded (1024 rows per core)
  - W_dec.T column-sharded (each core produces 512 output elements)
  - query replicated
One AllGather exchanges each core's local top-8 candidate rows + scores; every
core then reduces the 64 candidates to the global top-8, means the winning
rows and decodes its own output slice.

dtype notes: fp32r (full-rate PE fp32) is only legal when the operand is DMA'd
directly from an fp32r DRAM tensor, so mt/qcol/wt are declared fp32r and the
decode lhsT bounces through DRAM. Row sum-of-squares uses bf16 squares (errors
average out over the 4096-term fp32 PSUM accumulation).
"""

import numpy as np

import concourse.bass as bass
import concourse.mybir as mybir
import concourse.tile as tile
from concourse import bacc
from concourse.bass import ts
from concourse.bass_utils import run_bass_kernel_spmd

DIM = 4096
CAP = 8192
NCORES = 8
RPC = CAP // NCORES   # rows per core        (1024)
OPC = DIM // NCORES   # output elems / core  (512)
K = 8                 # top_k
DC = DIM // 128       # d-chunks             (32)
H = RPC // 2          # psum half            (512)
CCW = K * DIM + K     # collective payload   (32776 floats)
EPS = 1e-8

F32 = mybir.dt.float32
F32R = mybir.dt.float32r
BF16 = mybir.dt.bfloat16
U32 = mybir.dt.uint32
AF = mybir.ActivationFunctionType


def _build_nc():
    nc = bacc.Bacc(None, num_devices=NCORES, debug=False)

    mt = nc.dram_tensor("mt", [DIM, RPC], F32R, kind="ExternalInput")
    mrow = nc.dram_tensor("mrow", [RPC, DIM], F32, kind="ExternalInput")
    qcol = nc.dram_tensor("qcol", [128, DC], F32R, kind="ExternalInput")
    impa = nc.dram_tensor("impa", [32, 32], F32, kind="ExternalInput")
    agev = nc.dram_tensor("agev", [32, 32], F32, kind="ExternalInput")
    wt = nc.dram_tensor("wt", [DIM, OPC], F32R, kind="ExternalInput")
    bcv = nc.dram_tensor("bcv", [1, OPC], F32, kind="ExternalInput")
    out = nc.dram_tensor("out", [1, OPC], F32, kind="ExternalOutput")

    with tile.TileContext(nc) as tc:
        with (
            tc.tile_pool(name="persist", bufs=1) as pp,
            tc.tile_pool(name="mtp", bufs=3) as mtp,
            tc.tile_pool(name="sqp", bufs=3) as sqp,
            tc.tile_pool(name="small", bufs=1) as sp,
            tc.tile_pool(name="psum", bufs=1, space="PSUM") as psp,
            tc.tile_pool(name="dram", bufs=1, space="DRAM") as dp,
        ):
            # ---- constants
            ones_bf = pp.tile([128, 1], BF16, name="ones_bf")
            nc.vector.memset(ones_bf, 1.0)
            ones = pp.tile([128, 1], F32, name="ones")
            nc.vector.memset(ones, 1.0)
            eighth = pp.tile([8, 1], F32, name="eighth")
            nc.vector.memset(eighth, 1.0 / K)

            # ---- query columns [128, 32] (d-chunk c in column c)
            qt = pp.tile([128, DC], F32R, name="qt")
            nc.sync.dma_start(qt, qcol[:, :])

            # ---- ||q||^2
            qsq = sp.tile([128, DC], F32, name="qsq")
            qpart = sp.tile([128, 1], F32, name="qpart")
            nc.scalar.activation(qsq, qt.bitcast(F32), AF.Square, accum_out=qpart)
            qn2_ps = psp.tile([1, 1], F32, name="qn2_ps", tag="pB")
            nc.tensor.matmul(qn2_ps, lhsT=qpart, rhs=ones, start=True, stop=True)
            qn2 = sp.tile([1, 1], F32, name="qn2")
            nc.vector.tensor_copy(qn2, qn2_ps)
            qn2b = sp.tile([32, 1], F32, name="qn2b")
            nc.gpsimd.partition_broadcast(qn2b, qn2)

            # ---- ie = importance * exp(-0.001 * age)   [32, 32]
            imp_sb = sp.tile([32, 32], F32, name="imp_sb")
            nc.sync.dma_start(imp_sb, impa[:, :])
            age_sb = sp.tile([32, 32], F32, name="age_sb")
            nc.sync.dma_start(age_sb, agev[:, :])
            ie = sp.tile([32, 32], F32, name="ie")
            nc.scalar.activation(ie, age_sb, AF.Exp, scale=-0.001)
            nc.vector.tensor_mul(ie, ie, imp_sb)

            # ---- prefetch decoder slice [128, 32, 512]
            wt_sb = pp.tile([128, DC, OPC], F32R, name="wt_sb")
            for c in range(DC):
                nc.sync.dma_start(wt_sb[:, c, :], wt[ts(c, 128), :])

            # ---- phase A: dots = M @ q (f32r) and row sum-of-squares (bf16)
            dots_ps = [
                psp.tile([1, H], F32, name=f"dots_ps{h}", tag=f"dots{h}")
                for h in range(2)
            ]
            ssq_ps = [
                psp.tile([1, H], F32, name=f"ssq_ps{h}", tag=f"ssq{h}")
                for h in range(2)
            ]
            for t in range(DC):
                mt_t = mtp.tile([128, RPC], F32R, name="mt_t", tag="mt")
                nc.sync.dma_start(mt_t, mt[ts(t, 128), :])
                lq = qt[:, t : t + 1]
                for h in range(2):
                    nc.tensor.matmul(
                        dots_ps[h],
                        lhsT=lq,
                        rhs=mt_t[:, ts(h, H)],
                        start=(t == 0),
                        stop=(t == DC - 1),
                    )
                sq_t = sqp.tile([128, RPC], BF16, name="sq_t", tag="sq")
                nc.scalar.square(sq_t, mt_t.bitcast(F32))
                for h in range(2):
                    nc.tensor.matmul(
                        ssq_ps[h],
                        lhsT=ones_bf,
                        rhs=sq_t[:, ts(h, H)],
                        start=(t == 0),
                        stop=(t == DC - 1),
                    )

            # ---- phase B: scores + local top-8
            dflat = sp.tile([1, RPC], F32, name="dflat")
            ssflat = sp.tile([1, RPC], F32, name="ssflat")
            for h in range(2):
                nc.vector.tensor_copy(dflat[:, ts(h, H)], dots_ps[h])
                nc.scalar.copy(ssflat[:, ts(h, H)], ssq_ps[h])
            d32 = sp.tile([32, 32], F32, name="d32")
            nc.sync.dma_start(d32, dflat)
            ss32 = sp.tile([32, 32], F32, name="ss32")
            nc.sync.dma_start(ss32, ssflat)

            # den = max(sqrt(ssq * qn2), eps); s = dots / den * ie
            nc.vector.tensor_scalar_mul(ss32, ss32, qn2b)
            nc.scalar.sqrt(ss32, ss32)
            nc.vector.tensor_scalar_max(ss32, ss32, EPS)
            rden = sp.tile([32, 32], F32, name="rden")
            nc.vector.reciprocal(rden, ss32)
            s32 = sp.tile([32, 32], F32, name="s32")
            nc.vector.tensor_mul(s32, d32, rden)
            nc.vector.tensor_mul(s32, s32, ie)

            sflat = sp.tile([1, RPC], F32, name="sflat")
            nc.sync.dma_start(sflat, s32)
            mx8 = sp.tile([1, 8], F32, name="mx8")
            nc.vector.max(out=mx8, in_=sflat)
            idx8 = sp.tile([1, 8], U32, name="idx8")
            nc.vector.max_index(out=idx8, in_max=mx8, in_values=sflat)
            idxc = sp.tile([8, 1], U32, name="idxc")
            nc.sync.dma_start(idxc, idx8)

            # ---- gather local top-8 rows
            rows8 = sp.tile([8, DIM], F32, name="rows8")
            nc.gpsimd.indirect_dma_start(
                out=rows8[:],
                out_offset=None,
                in_=mrow[:, :],
                in_offset=bass.IndirectOffsetOnAxis(ap=idxc[:, :1], axis=0),
            )

            # ---- AllGather candidates (rows + vals)
            cc_in = dp.tile([CCW], F32, name="cc_in")
            cc_out = dp.tile([NCORES * CCW], F32, name="cc_out", addr_space="Shared")
            nc.sync.dma_start(
                cc_in[: K * DIM].rearrange("(r d) -> r d", d=DIM), rows8
            )
            nc.sync.dma_start(cc_in[K * DIM :].unsqueeze(0), mx8)
            nc.gpsimd.collective_compute(
                "AllGather",
                mybir.AluOpType.bypass,
                replica_groups=[list(range(NCORES))],
                ins=[cc_in.opt()],
                outs=[cc_out.opt()],
            )
            cc8 = cc_out.rearrange("(c x) -> c x", x=CCW)

            # ---- global top-8 among the 64 candidates
            vals64 = sp.tile([1, 64], F32, name="vals64")
            nc.sync.dma_start(vals64, cc8[:, K * DIM :])
            gv8 = sp.tile([1, 8], F32, name="gv8")
            nc.vector.max(out=gv8, in_=vals64)
            gpos = sp.tile([1, 8], U32, name="gpos")
            nc.vector.max_index(out=gpos, in_max=gv8, in_values=vals64)
            gposc = sp.tile([8, 1], U32, name="gposc")
            nc.sync.dma_start(gposc, gpos)
            gposf = sp.tile([8, 1], F32, name="gposf")
            nc.vector.tensor_copy(gposf, gposc)

            # one-hot weights w64[j] = 1/8 if candidate j is a winner else 0
            iotaf = sp.tile([8, 64], F32, name="iotaf")
            nc.gpsimd.iota(
                iotaf,
                pattern=[[1, 64]],
                channel_multiplier=0,
                allow_small_or_imprecise_dtypes=True,
            )
            eqf = sp.tile([8, 64], F32, name="eqf")
            nc.vector.tensor_scalar(
                eqf, iotaf, gposf, None, op0=mybir.AluOpType.is_equal
            )
            w64_ps = psp.tile([64, 1], F32, name="w64_ps", tag="pB")
            nc.tensor.matmul(w64_ps, lhsT=eqf, rhs=eighth, start=True, stop=True)
            w64 = sp.tile([64, 1], F32, name="w64")
            nc.vector.tensor_copy(w64, w64_ps)

            # ---- retrieved = w64 . rows64, produced directly in [128, 32] layout
            rows64 = pp.tile([64, DIM], F32, name="rows64")
            for c in range(NCORES):
                nc.sync.dma_start(
                    rows64[ts(c, K), :],
                    cc8[c, : K * DIM].rearrange("(r d) -> r d", d=DIM),
                )
            ret_ps = psp.tile([128, DC], F32, name="ret_ps", tag="pA")
            for c in range(DC):
                nc.tensor.matmul(
                    ret_ps[:, c : c + 1],
                    lhsT=rows64[:, ts(c, 128)],
                    rhs=w64,
                    start=True,
                    stop=True,
                )
            ret = sp.tile([128, DC], F32, name="ret")
            nc.vector.tensor_copy(ret, ret_ps)
            # bounce through DRAM so the decode lhsT is a legal f32r operand
            rscr = dp.tile([128, DC], F32R, name="rscr")
            nc.sync.dma_start(rscr, ret.bitcast(F32R))
            ret_r = sp.tile([128, DC], F32R, name="ret_r")
            nc.sync.dma_start(ret_r, rscr[:, :])

            # ---- decode: out_slice = retrieved @ W_dec[slice].T + b[slice]
            out_ps = psp.tile([1, OPC], F32, name="out_ps", tag="pout")
            for c in range(DC):
                nc.tensor.matmul(
                    out_ps,
                    lhsT=ret_r[:, c : c + 1],
                    rhs=wt_sb[:, c, :],
                    start=(c == 0),
                    stop=(c == DC - 1),
                )
            bc_sb = sp.tile([1, OPC], F32, name="bc_sb")
            nc.sync.dma_start(bc_sb, bcv[:, :])
            out_sb = sp.tile([1, OPC], F32, name="out_sb")
            nc.vector.tensor_add(out_sb, out_ps, bc_sb)
            nc.sync.dma_start(out[:, :], out_sb)

    nc.compile()
    return nc


_NC_CACHE = {}


def _get_nc():
    if "nc" not in _NC_CACHE:
        _NC_CACHE["nc"] = _build_nc()
    return _NC_CACHE["nc"]


def _make_in_maps(query, memory_bank, importance, age, W_dec, b_dec):
    query = np.ascontiguousarray(np.asarray(query, dtype=np.float32))
    memory_bank = np.ascontiguousarray(np.asarray(memory_bank, dtype=np.float32))
    importance = np.ascontiguousarray(np.asarray(importance, dtype=np.float32))
    age = np.ascontiguousarray(np.asarray(age, dtype=np.float32))
    W_dec = np.ascontiguousarray(np.asarray(W_dec, dtype=np.float32))
    b_dec = np.ascontiguousarray(np.asarray(b_dec, dtype=np.float32))

    qcol = np.ascontiguousarray(query.reshape(DC, 128).T)  # [128, 32]
    in_maps = []
    for c in range(NCORES):
        rs = slice(c * RPC, (c + 1) * RPC)
        os = slice(c * OPC, (c + 1) * OPC)
        shard = memory_bank[rs]
        in_maps.append(
            {
                "mt": np.ascontiguousarray(shard.T),
                "mrow": np.ascontiguousarray(shard),
                "qcol": qcol,
                "impa": np.ascontiguousarray(importance[rs].reshape(32, 32)),
                "agev": np.ascontiguousarray(age[rs].reshape(32, 32)),
                "wt": np.ascontiguousarray(W_dec[os, :].T),
                "bcv": np.ascontiguousarray(b_dec[os].reshape(1, OPC)),
            }
        )
    return in_maps


def run(inputs, trace=False, **run_kwargs):
    """Build (cached), run on 8 cores, gather. Returns (output, BassKernelResults)."""
    assert int(inputs.get("top_k", K)) == K
    nc = _get_nc()
    in_maps = _make_in_maps(
        inputs["query"],
        inputs["memory_bank"],
        inputs["importance"],
        inputs["age"],
        inputs["W_dec"],
        inputs["b_dec"],
    )
    res = run_bass_kernel_spmd(
        nc, in_maps, core_ids=list(range(NCORES)), trace=trace, **run_kwargs
    )
    out = np.concatenate(
        [res.results[c]["out"].reshape(OPC) for c in range(NCORES)]
    ).astype(np.float32)
    return out, res


def kernel(**inputs) -> np.ndarray:
    out, _ = run(inputs, trace=False)
    return out


# revision 11
# speedup vs baseline: 1.1982x; 1.1982x over previous
"""Trainium2 Bass kernel for nn_BiologicalMemory (retrieval_knn).

Full-input contract: kernel(**inputs) takes the complete unsharded inputs and
returns the complete [4096] output. Internally shards across 8 NeuronCores:
  - memory_bank / importance / age row-sharded (1024 rows per core)
  - W_dec.T column-sharded (each core produces 512 output elements)
  - query replicated (pre-broadcast to 128 partitions on host)
One AllGather exchanges each core's local top-8 candidate rows (bf16) +
scores (fp32); every core then reduces the 64 candidates to the global top-8,
means the winning rows and decodes its own output slice.

Phase A runs entirely on DVE (fused dot+reduce) and ACT (square+accumulate)
from the natural row-major layout, so it is DMA-bound; the PE only does the
small tail matmuls. fp32r (full-rate PE fp32) is only legal when the operand
is DMA'd directly from DRAM, so wt is declared fp32r and the decode lhsT
bounces through DRAM scratch.
"""

import numpy as np

import concourse.bass as bass
import concourse.mybir as mybir
import concourse.tile as tile
from concourse import bacc
from concourse.bass import ts
from concourse.bass_utils import run_bass_kernel_spmd
from concourse.masks import make_identity

DIM = 4096
CAP = 8192
NCORES = 8
RPC = CAP // NCORES   # rows per core        (1024)
OPC = DIM // NCORES   # output elems / core  (512)
K = 8                 # top_k
NT = RPC // 128       # row tiles per core   (8)
DC = DIM // 128       # d-chunks             (32)
CCB = K * DIM + 2 * K  # collective payload in bf16 elems (rows + fp32 vals)
EPS = 1e-8

F32 = mybir.dt.float32
F32R = mybir.dt.float32r
BF16 = mybir.dt.bfloat16
U32 = mybir.dt.uint32
AF = mybir.ActivationFunctionType
ALU = mybir.AluOpType


CC_BF16 = True      # bf16 candidate rows in the AllGather payload
MULTI_Q = True      # spread rows64 reload over sync/scalar/gpsimd DMA queues
STAGE = 4           # 1=phase A, 2=+top8/gather, 3=+collective, 4=full (debug aid)





def _build_nc():
    return _build_nc_inner()


def _build_nc_inner():
    nc = bacc.Bacc(None, num_devices=NCORES, debug=False)
    _emit(nc)
    nc.compile()
    return nc


def _emit(nc):
    cc_dt = BF16 if CC_BF16 else F32
    ccb = K * DIM + (2 * K if CC_BF16 else K)

    mrow = nc.dram_tensor("mrow", [RPC, DIM], F32, kind="ExternalInput")
    qb_d = nc.dram_tensor("qb", [128, DIM], F32, kind="ExternalInput")
    impa = nc.dram_tensor("impa", [128, NT], F32, kind="ExternalInput")
    agev = nc.dram_tensor("agev", [128, NT], F32, kind="ExternalInput")
    wt = nc.dram_tensor("wt", [DIM, OPC], F32R, kind="ExternalInput")
    bcv = nc.dram_tensor("bcv", [1, OPC], F32, kind="ExternalInput")
    out = nc.dram_tensor("out", [1, OPC], F32, kind="ExternalOutput")

    with tile.TileContext(nc) as tc:
        with (
            tc.tile_pool(name="persist", bufs=1) as pp,
            tc.tile_pool(name="mtp", bufs=2) as mtp,
            tc.tile_pool(name="scr", bufs=2) as scrp,
            tc.tile_pool(name="small", bufs=1) as sp,
            tc.tile_pool(name="psum", bufs=1, space="PSUM") as psp,
            tc.tile_pool(name="dram", bufs=1, space="DRAM") as dp,
        ):
            # ---- broadcast query (each partition holds the full q)
            qb = pp.tile([128, DIM], F32, name="qb")
            nc.gpsimd.dma_start(qb, qb_d[:, :])

            ident = pp.tile([128, 128], F32, name="ident")
            make_identity(nc, ident)
            eighth = pp.tile([8, 1], F32, name="eighth")
            nc.vector.memset(eighth, 1.0 / K)

            # ---- ||q||^2 on every partition (qb rows are all q)
            qscr = scrp.tile([128, DIM], BF16, name="qscr", tag="actscr")
            qn2col = sp.tile([128, 1], F32, name="qn2col")
            nc.scalar.activation(qscr, qb, AF.Square, accum_out=qn2col)

            # ---- ie = importance * exp(-0.001 * age)   [128, 8]
            imp_sb = sp.tile([128, NT], F32, name="imp_sb")
            nc.gpsimd.dma_start(imp_sb, impa[:, :])
            age_sb = sp.tile([128, NT], F32, name="age_sb")
            nc.gpsimd.dma_start(age_sb, agev[:, :])
            ie8 = sp.tile([128, NT], F32, name="ie8")
            nc.scalar.activation(ie8, age_sb, AF.Exp, scale=-0.001)
            nc.vector.tensor_mul(ie8, ie8, imp_sb)

            # ---- phase A: per row-tile, dots (DVE fused mul+reduce) and
            #      sum-of-squares (ACT square+accumulate)
            dots8 = sp.tile([128, NT], F32, name="dots8")
            ss8 = sp.tile([128, NT], F32, name="ss8")
            for t in range(NT):
                m_t = mtp.tile([128, DIM], F32, name="m_t", tag="m")
                nc.sync.dma_start(m_t, mrow[ts(t, 128), :])
                dscr = scrp.tile([128, DIM], BF16, name="dscr", tag="dvescr")
                nc.vector.affine_mul_reduce(
                    out=dscr,
                    accum_out=dots8[:, t : t + 1],
                    in0=m_t,
                    in1=qb,
                    scale=1.0,
                    bias=0.0,
                )
                ascr = scrp.tile([128, DIM], BF16, name="ascr", tag="actscr")
                nc.scalar.activation(
                    ascr, m_t, AF.Square, accum_out=ss8[:, t : t + 1]
                )

            if STAGE in (11, 12, 13):
                out_sbx = sp.tile([1, OPC], F32, name="out_sbx")
                nc.vector.memset(out_sbx, 0.0)
                if STAGE == 11:
                    nc.vector.tensor_copy(out_sbx[:, :NT], dots8[0:1, :])
                elif STAGE == 12:
                    nc.vector.tensor_copy(out_sbx[:, :NT], ss8[0:1, :])
                else:
                    tmp13 = sp.tile([128, NT], F32, name="tmp13")
                    nc.vector.tensor_scalar_mul(tmp13, ss8, qn2col)
                    nc.vector.tensor_mul(tmp13, tmp13, dots8)
                    nc.vector.tensor_copy(out_sbx[:, :NT], tmp13[0:1, :])
                nc.sync.dma_start(out[:, :], out_sbx)
                return

            # ---- decoder slice prefetch (after phase A traffic; overlaps CC)
            wt_sb = pp.tile([128, DC, OPC], F32R, name="wt_sb")
            for c in range(DC):
                nc.gpsimd.dma_start(wt_sb[:, c, :], wt[ts(c, 128), :])

            # ---- scores [128, 8]: s = dots / max(sqrt(ssq*qn2), eps) * ie
            den = sp.tile([128, NT], F32, name="den")
            nc.vector.tensor_scalar_mul(den, ss8, qn2col)
            nc.scalar.sqrt(den, den)
            nc.vector.tensor_scalar_max(den, den, EPS)
            rden = sp.tile([128, NT], F32, name="rden")
            nc.vector.reciprocal(rden, den)
            s8 = sp.tile([128, NT], F32, name="s8")
            nc.vector.tensor_mul(s8, dots8, rden)
            nc.vector.tensor_mul(s8, s8, ie8)

            # ---- flatten scores to [1, 1024] in row order (r = t*128 + p)
            st_ps = psp.tile([NT, 128], F32, name="st_ps", tag="pT")
            nc.tensor.transpose(st_ps, s8, ident)
            st = sp.tile([NT, 128], F32, name="st")
            nc.vector.tensor_copy(st, st_ps)
            sflat = sp.tile([1, RPC], F32, name="sflat")
            nc.sync.dma_start(sflat, st)

            if STAGE == 1:
                out_sb1 = sp.tile([1, OPC], F32, name="out_sb1")
                nc.vector.tensor_copy(out_sb1, sflat[:, :OPC])
                nc.sync.dma_start(out[:, :], out_sb1)
                return

            # ---- local top-8
            mx8 = sp.tile([1, 8], F32, name="mx8")
            nc.vector.max(out=mx8, in_=sflat)
            idx8 = sp.tile([1, 8], U32, name="idx8")
            nc.vector.max_index(out=idx8, in_max=mx8, in_values=sflat)
            idxc = sp.tile([8, 1], U32, name="idxc")
            nc.sync.dma_start(idxc, idx8)

            # ---- gather local top-8 rows, cast to bf16 for transport
            rows8 = sp.tile([8, DIM], F32, name="rows8")
            nc.gpsimd.indirect_dma_start(
                out=rows8[:],
                out_offset=None,
                in_=mrow[:, :],
                in_offset=bass.IndirectOffsetOnAxis(ap=idxc[:, :1], axis=0),
            )
            if CC_BF16:
                rows8_cc = sp.tile([8, DIM], BF16, name="rows8_bf")
                nc.vector.tensor_copy(rows8_cc, rows8)
            else:
                rows8_cc = rows8

            if STAGE == 2:
                out_sb2 = sp.tile([1, OPC], F32, name="out_sb2")
                nc.vector.tensor_copy(out_sb2, rows8[0:1, :OPC])
                nc.sync.dma_start(out[:, :], out_sb2)
                return

            # ---- AllGather candidates (rows + fp32 vals bit-packed)
            cc_in = dp.tile([ccb], cc_dt, name="cc_in")
            cc_out = dp.tile([NCORES * ccb], cc_dt, name="cc_out", addr_space="Shared")
            nc.sync.dma_start(
                cc_in[: K * DIM].rearrange("(r d) -> r d", d=DIM), rows8_cc
            )
            nc.sync.dma_start(
                cc_in[K * DIM :].unsqueeze(0),
                mx8.bitcast(BF16) if CC_BF16 else mx8,
            )
            nc.gpsimd.collective_compute(
                "AllGather",
                ALU.bypass,
                replica_groups=[list(range(NCORES))],
                ins=[cc_in.opt()],
                outs=[cc_out.opt()],
            )
            cc8 = cc_out.rearrange("(c x) -> c x", x=ccb)

            # ---- global top-8 among the 64 candidates
            vals64 = sp.tile([1, 64], F32, name="vals64")
            nc.sync.dma_start(vals64, cc8[:, K * DIM :].bitcast(F32) if CC_BF16 else cc8[:, K * DIM :])
            gv8 = sp.tile([1, 8], F32, name="gv8")
            nc.vector.max(out=gv8, in_=vals64)
            gpos = sp.tile([1, 8], U32, name="gpos")
            nc.vector.max_index(out=gpos, in_max=gv8, in_values=vals64)
            gposc = sp.tile([8, 1], U32, name="gposc")
            nc.sync.dma_start(gposc, gpos)
            gposf = sp.tile([8, 1], F32, name="gposf")
            nc.vector.tensor_copy(gposf, gposc)

            if STAGE == 3:
                out_sb3 = sp.tile([1, OPC], F32, name="out_sb3")
                nc.vector.memset(out_sb3, 0.0)
                nc.vector.tensor_copy(out_sb3[:, :64], vals64)
                nc.sync.dma_start(out[:, :], out_sb3)
                return

            # one-hot weights w64[j] = 1/8 if candidate j is a winner else 0
            iotaf = sp.tile([8, 64], F32, name="iotaf")
            nc.gpsimd.iota(
                iotaf,
                pattern=[[1, 64]],
                channel_multiplier=0,
                allow_small_or_imprecise_dtypes=True,
            )
            eqf = sp.tile([8, 64], F32, name="eqf")
            nc.vector.tensor_scalar(eqf, iotaf, gposf, None, op0=ALU.is_equal)
            w64_ps = psp.tile([64, 1], F32, name="w64_ps", tag="pB")
            nc.tensor.matmul(w64_ps, lhsT=eqf, rhs=eighth, start=True, stop=True)
            w64_bf = sp.tile([64, 1], cc_dt, name="w64_bf")
            nc.vector.tensor_copy(w64_bf, w64_ps)

            # ---- candidate rows [64, 4096] bf16 (spread across DMA queues)
            rows64 = pp.tile([64, DIM], cc_dt, name="rows64")
            dma_engines = [nc.sync, nc.scalar, nc.gpsimd] if MULTI_Q else [nc.sync]
            for c in range(NCORES):
                dma_engines[c % len(dma_engines)].dma_start(
                    rows64[ts(c, K), :],
                    cc8[c, : K * DIM].rearrange("(r d) -> r d", d=DIM),
                )

            # ---- retrieved = w64 . rows64, produced directly in [128, 32] layout
            ret_ps = psp.tile([128, DC], F32, name="ret_ps", tag="pA")
            for c in range(DC):
                nc.tensor.matmul(
                    ret_ps[:, c : c + 1],
                    lhsT=rows64[:, ts(c, 128)],
                    rhs=w64_bf,
                    start=True,
                    stop=True,
                )
            ret = sp.tile([128, DC], F32, name="ret")
            nc.vector.tensor_copy(ret, ret_ps)
            # bounce through DRAM so the decode lhsT is a legal f32r operand
            rscr = dp.tile([128, DC], F32R, name="rscr")
            nc.sync.dma_start(rscr, ret.bitcast(F32R))
            ret_r = sp.tile([128, DC], F32R, name="ret_r")
            nc.sync.dma_start(ret_r, rscr[:, :])

            # ---- decode: out_slice = retrieved @ W_dec[slice].T + b[slice]
            out_ps = psp.tile([1, OPC], F32, name="out_ps", tag="pout")
            for c in range(DC):
                nc.tensor.matmul(
                    out_ps,
                    lhsT=ret_r[:, c : c + 1],
                    rhs=wt_sb[:, c, :],
                    start=(c == 0),
                    stop=(c == DC - 1),
                )
            bc_sb = sp.tile([1, OPC], F32, name="bc_sb")
            nc.gpsimd.dma_start(bc_sb, bcv[:, :])
            out_sb = sp.tile([1, OPC], F32, name="out_sb")
            nc.vector.tensor_add(out_sb, out_ps, bc_sb)
            nc.sync.dma_start(out[:, :], out_sb)


_NC_CACHE = {}


def _get_nc():
    if "nc" not in _NC_CACHE:
        _NC_CACHE["nc"] = _build_nc()
    return _NC_CACHE["nc"]


def _make_in_maps(query, memory_bank, importance, age, W_dec, b_dec):
    query = np.ascontiguousarray(np.asarray(query, dtype=np.float32))
    memory_bank = np.ascontiguousarray(np.asarray(memory_bank, dtype=np.float32))
    importance = np.ascontiguousarray(np.asarray(importance, dtype=np.float32))
    age = np.ascontiguousarray(np.asarray(age, dtype=np.float32))
    W_dec = np.ascontiguousarray(np.asarray(W_dec, dtype=np.float32))
    b_dec = np.ascontiguousarray(np.asarray(b_dec, dtype=np.float32))

    qb = np.ascontiguousarray(np.broadcast_to(query[None, :], (128, DIM)))
    in_maps = []
    for c in range(NCORES):
        rs = slice(c * RPC, (c + 1) * RPC)
        os = slice(c * OPC, (c + 1) * OPC)
        in_maps.append(
            {
                "mrow": np.ascontiguousarray(memory_bank[rs]),
                "qb": qb,
                "impa": np.ascontiguousarray(importance[rs].reshape(NT, 128).T),
                "agev": np.ascontiguousarray(age[rs].reshape(NT, 128).T),
                "wt": np.ascontiguousarray(W_dec[os, :].T),
                "bcv": np.ascontiguousarray(b_dec[os].reshape(1, OPC)),
            }
        )
    return in_maps


def run(inputs, trace=False, **run_kwargs):
    """Build (cached), run on 8 cores, gather. Returns (output, BassKernelResults)."""
    assert int(inputs.get("top_k", K)) == K
    nc = _get_nc()
    in_maps = _make_in_maps(
        inputs["query"],
        inputs["memory_bank"],
        inputs["importance"],
        inputs["age"],
        inputs["W_dec"],
        inputs["b_dec"],
    )
    res = run_bass_kernel_spmd(
        nc, in_maps, core_ids=list(range(NCORES)), trace=trace, **run_kwargs
    )
    out = np.concatenate(
        [res.results[c]["out"].reshape(OPC) for c in range(NCORES)]
    ).astype(np.float32)
    return out, res


def kernel(**inputs) -> np.ndarray:
    out, _ = run(inputs, trace=False)
    return out
